# revision 50
# baseline (speedup 1.0000x reference)
"""Trainium2 Bass kernel for causal self-attention (B=2, S=2048, D=1024, H=16).

Sharding: 8 cores = 2 batches x 4 head-groups. Core c handles batch c//4 and
heads 4*(c%4) .. 4*(c%4)+4. Each core receives its batch's x (transposed,
fp16, pre-chunked) and its slice of w_qkv (q/k column block and v block as
separate contiguous tensors), and produces the [2048, 256] output slice. No
cross-core communication is needed; the host gathers the slices. w_o is
unused by the reference (no output projection).

Per-core kernel (Tile framework), fp16 matmul path with fp32 psum/softmax.
Software-pipelined emission as one continuous slot stream over all (chunk,
head-pair, j-block) score/exp/AV steps: the next slot's score matmul is
always ahead of the 2-slots-delayed AV matmuls, projection chains for later
chunks drain between attention matmuls, and each pair's finalize runs inside
the next pair's early slots. Exp work on the diagonal j-blocks is trimmed to
the unmasked column range via strided APs. Operand tensors are split into
per-chunk tiles so the Tile dependency tracker never serializes a consumer
behind an unrelated later write to the same tile.
"""

import sys

sys.path.insert(0, "/opt/trn_rl_repo")

from collections import deque
from contextlib import ExitStack

import numpy as np

import concourse.bass as bass
import concourse.tile as tile
from concourse import bacc, masks, mybir
from concourse.bass_utils import run_bass_kernel_spmd

B, S, D, H = 2, 2048, 1024, 16
HD = 64          # head dim
HPC = 4          # heads per core
NCORES = 8
P = 128
NS = S // P      # 16 s-blocks
KC = D // P      # 8 d-chunks
CH = 512         # query-chunk width
NT = S // CH     # 4 query chunks
F32 = mybir.dt.float32
F16 = mybir.dt.float16
SCALE = 1.0 / np.sqrt(HD)

PSUM = bass.MemorySpace.PSUM


def _build_body(ctx: ExitStack, tc: "tile.TileContext", x_d, wqk_d, wv_d, o_d):
    nc = tc.nc

    persist = ctx.enter_context(tc.tile_pool(name="persist", bufs=1))
    ident_h = persist.tile([P, P], F16)
    masks.make_identity(nc, ident_h[:])

    # per-s-block v in natural layout + ones column: [128, 4 heads x 65]
    vts = [persist.tile([P, HPC * 65], F16, name=f"v{sb}") for sb in range(NS)]
    for sb in range(NS):
        nc.vector.memset(
            vts[sb][:].rearrange("p (n c) -> p n c", c=65)[:, :, 64:65], 1.0
        )
    # final output staging [128, 16 i-blocks * 4 heads * 64]
    out_sb = persist.tile([P, NS * HPC * HD], F32)
    # weights: q+k column blocks (m0=q01, m1=q23, m2=k01, m3=k23), v separate
    w_qk = persist.tile([P, KC * 512], F16)
    w_v = persist.tile([P, KC * 256], F16)
    # x^T per query chunk [128, 8 d-chunks x 512]
    xts = [persist.tile([P, KC * CH], F16, name=f"x{t}") for t in range(NT)]
    # projected q/k, per (m, chunk): [128, 512] each
    qk = [[persist.tile([P, CH], F16, name=f"qk{m}_{t}") for t in range(NT)]
          for m in range(4)]

    # Force the exp table load to happen during kernel init, before the
    # scalar queue picks up its share of the input DMAs.
    actwarm = persist.tile([1, 1], F32)
    nc.scalar.activation(
        actwarm[:], ident_h[0:1, 0:1], mybir.ActivationFunctionType.Exp
    )

    # ---- all input DMAs up front ----------------------------------------
    # All source tensors are host-packed so every DMA reads a fully
    # contiguous DRAM block. Ramp-critical data is interleaved across both
    # HWDGE queues: sync gets the q/k weight blocks, scalar gets x chunk 0,
    # so the first projection chains' k-th matmul unblocks after k+1 DMAs
    # on each queue.
    for k in range(KC):
        nc.sync.dma_start(
            w_qk[:, k * 512:(k + 1) * 512],
            wqk_d[k * P:(k + 1) * P, :],
        )
        nc.scalar.dma_start(
            xts[0][:, k * CH:(k + 1) * CH],
            x_d[0, k * P:(k + 1) * P, :],
        )
    # w_v rides the gpsimd software-DGE queue (2 merged transfers) so that
    # neither HWDGE queue's completion counter gates the prologue matmuls
    # on it, and the scalar queue stays clear for the exp stream.
    for h in range(2):
        nc.gpsimd.dma_start(
            w_v[:, h * 4 * 256:(h + 1) * 4 * 256]
            .rearrange("p (k c) -> p k c", c=256),
            wv_d[h * 4 * P:(h + 1) * 4 * P, :]
            .rearrange("(k p) c -> p k c", p=P),
        )
    for t in range(1, NT):
        for k in range(KC):
            nc.sync.dma_start(
                xts[t][:, k * CH:(k + 1) * CH],
                x_d[t, k * P:(k + 1) * P, :],
            )

    ps_st = ctx.enter_context(tc.tile_pool(name="ps_st", bufs=2, space=PSUM))
    ps_o = ctx.enter_context(tc.tile_pool(name="ps_o", bufs=2, space=PSUM))
    ps_small = ctx.enter_context(tc.tile_pool(name="ps_small", bufs=2, space=PSUM))
    pp = ctx.enter_context(tc.tile_pool(name="p", bufs=6))
    osbp = ctx.enter_context(tc.tile_pool(name="osb", bufs=4))
    rcp = ctx.enter_context(tc.tile_pool(name="rcol", bufs=4))



    # ---- background work: projection chains, emitted in small units ------
    # Each chain is split into units of 4 matmuls; the last unit also emits
    # the PSUM->SBUF cast. Units are drained between attention matmuls so
    # the PE has independent work while the scalar engine runs exp.
    bg = deque()          # of (emit_fn, chain_name, is_last_unit)
    done = set()

    def add_chain(name, units):
        for i, u in enumerate(units):
            bg.append((u, name, i == len(units) - 1))

    def drain(n):
        for _ in range(n):
            if not bg:
                return
            u, nm, last = bg.popleft()
            u()
            if last:
                done.add(nm)

    def require(name):
        while name not in done:
            assert bg, f"chain {name} needed but not queued"
            u, nm, last = bg.popleft()
            u()
            if last:
                done.add(nm)

    def proj_chain(m, t):
        """qk[m][t] = (w col-block m)^T @ x^T chunk t, as 2 units."""
        st = {}

        def u1():
            st["ps"] = ps_small.tile([P, CH], F32, tag="small", name="pp_ps")
            for k in range(4):
                nc.tensor.matmul(
                    st["ps"][:],
                    w_qk[:, k * 512 + m * P: k * 512 + (m + 1) * P],
                    xts[t][:, k * CH:(k + 1) * CH],
                    start=(k == 0),
                    stop=False,
                )

        def u2():
            for k in range(4, KC):
                nc.tensor.matmul(
                    st["ps"][:],
                    w_qk[:, k * 512 + m * P: k * 512 + (m + 1) * P],
                    xts[t][:, k * CH:(k + 1) * CH],
                    start=False,
                    stop=(k == KC - 1),
                )
            nc.vector.tensor_copy(qk[m][t][:], st["ps"][:])

        return [u1, u2]

    def v_chain(sb):
        """vts[sb] = x[s-block sb] @ w_v (natural layout), as 2 units."""
        st = {}
        t, o = sb // 4, (sb % 4) * P

        def u1():
            st["ps"] = ps_small.tile([P, CH], F32, tag="small", name="pv")
            for k in range(4):
                nc.tensor.matmul(
                    st["ps"][:, 0:256],
                    xts[t][:, k * CH + o: k * CH + o + P],
                    w_v[:, k * 256:(k + 1) * 256],
                    start=(k == 0),
                    stop=False,
                )

        def u2():
            for k in range(4, KC):
                nc.tensor.matmul(
                    st["ps"][:, 0:256],
                    xts[t][:, k * CH + o: k * CH + o + P],
                    w_v[:, k * 256:(k + 1) * 256],
                    start=False,
                    stop=(k == KC - 1),
                )
            nc.vector.tensor_copy(
                vts[sb][:].rearrange("p (g c) -> p g c", c=65)[:, :, 0:64],
                st["ps"][:, 0:256].rearrange("p (g c) -> p g c", c=64),
            )

        return [u1, u2]

    out_view = out_sb[:].rearrange("p (i g d) -> p i g d", g=HPC, d=HD)

    def finalize_copy(po_h):
        """PSUM->SBUF copy of a head's accumulated outT; frees the po bank."""
        osb_t = osbp.tile([65, CH], F16, tag="osb")
        nc.vector.tensor_copy(osb_t[:], po_h[:])
        return osb_t

    def finalize_transform(h, t, osb_t):
        """Transpose outT to natural layout, divide by the denominator."""
        fin32 = ps_small.tile([P, CH], F32, tag="small", name="fin32")
        fin = fin32.bitcast(F16)[:, 0:CH]
        for b in range(4):
            nc.tensor.transpose(
                fin[:, b * P:b * P + 65],
                osb_t[:, b * P:(b + 1) * P],
                ident_h[0:65, 0:65],
            )
        fin_view = fin[:, 0:CH].rearrange("p (n c) -> p n c", c=P)
        rc = rcp.tile([P, 4], F32, tag="rc")
        nc.vector.reciprocal(rc[:], fin_view[:, :, 64])
        nc.vector.tensor_mul(
            out_view[:, 4 * t:4 * t + 4, h, :],
            fin_view[:, :, 0:64],
            rc[:].broadcast_to([P, 4, HD]),
        )

    def out_dma(t):
        for b in range(4):
            ib = 4 * t + b
            # split the final chunk's output across both queues to halve
            # the end-of-kernel DMA tail (the scalar queue is idle by then)
            eng = nc.scalar if (t == NT - 1 and b % 2) else nc.sync
            eng.dma_start(
                o_d[ib * P:(ib + 1) * P, :],
                out_sb[:, ib * HPC * HD:(ib + 1) * HPC * HD],
            )

    # HAM warmer: ~3.4us of back-to-back dummy matmuls on the identity while
    # the first input DMAs run, so the PE clock gate reaches 8/8 (2.4 GHz)
    # about when the first projection chain issues.
    for _ in range(2):
        ham_ps = ps_small.tile([P, P], F32, tag="small", name="ham_ps")
        for _ in range(16):
            nc.tensor.matmul(
                ham_ps[:], ident_h[:], ident_h[:], start=True, stop=True,
            )

    # ---- prologue: ALL of chunk 0's q/k projections, interleaved per
    # k-chunk so each matmul issues as soon as its (w, x) DMA pair lands.
    # The four chains accumulate into the two ps_st tiles (two banks each).
    st_q = ps_st.tile([P, 2 * CH], F32, tag="st", name="st_q")   # q01 | q23
    st_k = ps_st.tile([P, 2 * CH], F32, tag="st", name="st_k")   # k01 | k23
    for k in range(KC):
        for mi, m in enumerate((0, 1)):
            nc.tensor.matmul(
                st_q[:, mi * CH:(mi + 1) * CH],
                w_qk[:, k * 512 + m * P: k * 512 + (m + 1) * P],
                xts[0][:, k * CH:(k + 1) * CH],
                start=(k == 0), stop=(k == KC - 1),
            )
        for mi, m in enumerate((2, 3)):
            nc.tensor.matmul(
                st_k[:, mi * CH:(mi + 1) * CH],
                w_qk[:, k * 512 + m * P: k * 512 + (m + 1) * P],
                xts[0][:, k * CH:(k + 1) * CH],
                start=(k == 0), stop=(k == KC - 1),
            )
    nc.vector.tensor_copy(qk[0][0][:], st_q[:, 0:CH])
    nc.vector.tensor_copy(qk[1][0][:], st_q[:, CH:2 * CH])
    nc.vector.tensor_copy(qk[2][0][:], st_k[:, 0:CH])
    nc.vector.tensor_copy(qk[3][0][:], st_k[:, CH:2 * CH])
    done.update({"q0t0", "q1t0", "k0t0", "k1t0"})
    for sb in range(4):
        add_chain(f"v{sb}", v_chain(sb))

    # ---- the unified slot stream ----------------------------------------
    slots = [
        (t, pair, jb)
        for t in range(NT)
        for pair in (0, 1)
        for jb in range(4 * t + 4)
    ]
    pending = deque()   # (t, pair, jb, p_t, po, last)
    slow_fin = deque()  # transform/out-dma closures, trickled one per slot
    po_cur = None

    def emit_av(t, pair, jb, p_t, po, last):
        hA, hB = 2 * pair, 2 * pair + 1
        doff = jb - 4 * t
        off = max(0, P * doff)
        require(f"v{jb}")
        for hi, h in enumerate((hA, hB)):
            nc.tensor.matmul(
                po[h][:, off:CH],
                vts[jb][:, h * 65:(h + 1) * 65],
                p_t[:, hi * CH + off:(hi + 1) * CH],
                start=(jb == 0),
                stop=last,
            )

    def pop_av():
        t, pair, jb, p_t, po, last = pending.popleft()
        emit_av(t, pair, jb, p_t, po, last)
        if last:
            # free the po banks right away; defer the PE/DVE transform work
            # so it trickles into later slots instead of piling up between
            # the next pair's first scores
            for h in (2 * pair, 2 * pair + 1):
                osb_t = finalize_copy(po[h])
                slow_fin.append(
                    lambda h=h, t=t, o=osb_t: finalize_transform(h, t, o)
                )
            if pair == 1:
                slow_fin.append(lambda t=t: out_dma(t))

    scored = deque()    # (t, pair, jb, st, po) — scores run 1 slot ahead

    def emit_score(t, pair, jb):
        if pair == 0 and jb == 0:
            # populate this phase's chains + next phase's pair-0 q/k, in
            # rough deadline order so the per-slot drain stays just-in-time
            if t > 0:
                add_chain(f"q1t{t}", proj_chain(1, t))
                add_chain(f"k1t{t}", proj_chain(3, t))
                for sb in range(4 * t, 4 * t + 4):
                    add_chain(f"v{sb}", v_chain(sb))
            if t + 1 < NT:
                add_chain(f"q0t{t + 1}", proj_chain(0, t + 1))
                add_chain(f"k0t{t + 1}", proj_chain(2, t + 1))
        if jb == 0:
            require(f"q{pair}t{t}")
            po_a = ps_o.tile([65, CH], F32, tag="o")
            po_b = ps_o.tile([65, CH], F32, tag="o")
            po = {2 * pair: po_a, 2 * pair + 1: po_b}
        else:
            po = po_cur   # same pair as the slot currently in flight
        if jb >= 4 * t:
            require(f"k{pair}t{t}")
        st = ps_st.tile([P, 2 * CH], F32, tag="st")
        kt = qk[2 + pair][jb // 4]
        kc0 = (jb % 4) * P
        for hi, h in enumerate((2 * pair, 2 * pair + 1)):
            hb = (h % 2) * 64
            nc.tensor.matmul(
                st[:, hi * CH:(hi + 1) * CH],
                kt[hb:hb + 64, kc0:kc0 + P],
                qk[pair][t][hb:hb + 64, :],
                start=True,
                stop=True,
                tile_position=(hb, 0),
            )
        scored.append((t, pair, jb, st, po))

    emit_score(*slots[0])
    for i, (t, pair, jb) in enumerate(slots):
        njb = 4 * t + 4
        _t, _pair, _jb, st, po_cur = scored.popleft()
        assert (_t, _pair, _jb) == (t, pair, jb)
        p_t = pp.tile([P, 2 * CH], F16, tag="p")
        doff = jb - 4 * t
        off = max(0, P * doff)
        if off > 0:
            # skip exp of the fully-masked leading columns of both heads
            st_v = st[:].rearrange("p (h w) -> p h w", h=2)[:, :, off:CH]
            p_v = p_t[:].rearrange("p (h w) -> p h w", h=2)[:, :, off:CH]
        else:
            st_v, p_v = st[:], p_t[:]
        nc.scalar.activation(
            p_v, st_v, mybir.ActivationFunctionType.Exp, scale=float(SCALE)
        )
        if doff >= 0:
            # triangular mask on the diagonal 128x128 sub-block only; the
            # fully-masked zone is skipped by exp + AV column ranges.
            for hi in range(2):
                c0 = hi * CH + off
                sl = p_t[:, c0:c0 + P]
                nc.gpsimd.affine_select(
                    out=sl,
                    in_=sl,
                    compare_op=mybir.AluOpType.is_ge,
                    fill=0.0,
                    base=0,
                    channel_multiplier=-1,
                    pattern=[[1, P]],
                )
        if i + 1 < len(slots):
            emit_score(*slots[i + 1])
        drain(2 if t == 0 else 1)
        pending.append((t, pair, jb, p_t, po_cur, jb == njb - 1))
        if len(pending) > 2:
            pop_av()
        # once a new pair's first score/exp is in flight, flush the old
        # pair's remaining AVs so its po banks free (via finalize_copy)
        # a couple of slots before this pair's first AV needs them —
        # absorbing the cross-engine sem latency without stalling the
        # next exp behind the tail AVs
        while pending and (pending[0][0], pending[0][1]) != (t, pair):
            pop_av()
        if slow_fin:
            slow_fin.popleft()()
    while pending:
        pop_av()
    while slow_fin:
        slow_fin.popleft()()
    assert not bg, f"{len(bg)} background units left unemitted"


def build_program():
    nc = bacc.Bacc(
        "TRN2",
        target_bir_lowering=False,
        debug=False,
        enable_asserts=False,
    )
    x_d = nc.dram_tensor("x", [NT, D, CH], F16, kind="ExternalInput").ap()
    wqk_d = nc.dram_tensor("wqk", [D, 512], F16, kind="ExternalInput").ap()
    wv_d = nc.dram_tensor("wv", [D, 256], F16, kind="ExternalInput").ap()
    o_d = nc.dram_tensor("o", [S, HPC * HD], F32, kind="ExternalOutput").ap()

    with tile.TileContext(nc) as tc, ExitStack() as ctx:
        _build_body(ctx, tc, x_d, wqk_d, wv_d, o_d)
    nc.compile()
    return nc


_CACHE = {}


def _compiled():
    if "nc" not in _CACHE:
        _CACHE["nc"] = build_program()
    return _CACHE["nc"]


def make_in_maps(x, w_qkv):
    x = np.asarray(x, dtype=np.float32)
    w_qkv = np.asarray(w_qkv, dtype=np.float32)
    # one transpose+cast per batch; cores sharing a batch reuse the array
    xT16 = []
    for b in range(B):
        xt = x[b].T.astype(np.float16)          # [D, S]
        xT16.append(
            np.ascontiguousarray(
                xt.reshape(D, NT, CH).transpose(1, 0, 2)
            )                                    # [NT, D, CH], chunk-major
        )
    in_maps = []
    for c in range(NCORES):
        b = c // 4
        cs = (c % 4) * HPC * HD
        wqk = np.concatenate(
            [
                w_qkv[:, cs:cs + HPC * HD],
                w_qkv[:, D + cs:D + cs + HPC * HD],
            ],
            axis=1,
        )
        wv = w_qkv[:, 2 * D + cs:2 * D + cs + HPC * HD]
        in_maps.append(
            {
                "x": xT16[b],
                "wqk": np.ascontiguousarray(wqk).astype(np.float16),
                "wv": np.ascontiguousarray(wv).astype(np.float16),
            }
        )
    return in_maps


def gather_out(results):
    out = np.empty((B, S, D), np.float32)
    for c in range(NCORES):
        b = c // 4
        cs = (c % 4) * HPC * HD
        out[b][:, cs:cs + HPC * HD] = results[c]["o"]
    return out


def kernel(x, w_qkv, w_o=None, **_):
    nc = _compiled()
    res = run_bass_kernel_spmd(nc, make_in_maps(x, w_qkv), core_ids=list(range(NCORES)))
    return gather_out(res.results)
